# revision 30
# baseline (speedup 1.0000x reference)
"""GAT attention kernel for 8 trn2 NeuronCores (Bass/Tile), bf16 v7.

Math (restructured from the reference to avoid materializing h_j):
    wa1 = W @ a1, wa2 = W @ a2                      (host, once, O(F^2))
    s[n,k]  = x[n,k]·wa2 - 500*(1-adj[n,k])         (mask folded into score)
    s2      = s + si,  si = x0[n]·wa1
    e       = exp(leaky_relu(s2, 0.2))              (masked: exp(~-100) ~ 0,
                                                     same as reference -9e15 mask)
    att     = e / (sum_k e + 16*EPS)
    xbar[n] = sum_k att[n,k] * x[n,k,:]
    out     = elu((xbar + x0) @ W)                  (h_prime + h = (xbar + x0)@W)
    elu(z)  = min(exp(z) - 1, relu(z))

Layout: node dim N padded 50000 -> 51200 = 8 cores * 25 pairs * 256 rows.
TWO 128-row tiles are processed per pipeline step ("pair") to amortize the
~151-cycle DVE and ~222-cycle ACT fixed costs.  Per tile, 17 segments of 132
bf16 elems: 16x [x_block(128) | -500(1-adj) | 0 0 0] (s-layout nk blocks) +
[x0 | 0...].  Scores: one 2x TT product against a precomposed weights tile,
two 2x bf16 fold-adds (132->66->33), then the 1x segmented reduce on a third
of the data -- ~25% less DVE time than reducing directly.
Final matmul is transposed (lhsT = W, rhs = xbarT(+x0T) pair-wide): one
256-col matmul per pair; output lands [f_out, n] and the host transposes.

Per pair:
  DVE : prod TT, fold1, fold2, segmented reduce, attseg = SEGBIG*att,
        recip_fast, y = min(E-1, R)
  PE  : 2x si scatter (Cm), 2x Z (SEG), 2x RZrep (E8), 2x x0^T ident +
        32 xbar matmuls (PSUM), final W^T-form matmul (256 cols)
  ACT : si_s, Prelu, Exp, tz (+16eps), RZrep copy, ST copy, R, E
  GPS : 2x Dt (si bcast mask), s2 = s+si_s, att = e*RZ
"""

import numpy as np

N, K, F = 50000, 16, 128
ALPHA = 0.2
NCORES = 8
TILE = 128
NTILES = 50                  # padded to even
NPAIR = NTILES // 2
RPC = TILE * NTILES          # rows per core = 6400
BPT = K                      # nk-blocks per tile = 16
SEGL = F + 4                 # 128 features + mask bias + 3 pad (fold-friendly)
XCOLS = (BPT + 1) * SEGL     # 17 segments of 132 = 2244
XC2 = 2 * XCOLS              # pair row = 4488
EPS = 1e-12
MASKB = -500.0

_NC_CACHE = {}


def _consts_f32_np():
    p = np.arange(128)
    j8 = np.arange(8)
    b16 = np.arange(16)
    # C[n, q] = 1 iff n%8 == q//16   (si scatter: out[q,b] = si[8b + q//16])
    Cm = (p[:, None] % 8 == p[None, :] // 16).astype(np.float32)
    # SEG[q, j] = 1 iff q//16 == j   [128, 8]
    seg = (p[:, None] // 16 == j8[None, :]).astype(np.float32)
    # E8 rows 0..8: E8[j, q] = 1 iff q//16 == j (used as lhsT [8,128])
    e8 = ((p[:, None] < 8) & (p[None, :] // 16 == p[:, None])).astype(np.float32)
    # SEG8[n, b] = 1 iff n//8 == b   [128, 16], tiled x2 for the pair
    seg8 = (p[:, None] // 8 == b16[None, :]).astype(np.float32)
    seg8_2 = np.concatenate([seg8, seg8], axis=1)
    return np.ascontiguousarray(
        np.concatenate([Cm, seg, e8, seg8_2], axis=1))  # [128, 296]


def _consts_bf16_np(W, a):
    import ml_dtypes
    bf16 = ml_dtypes.bfloat16
    p = np.arange(128)
    W = np.asarray(W, np.float32)
    a = np.asarray(a, np.float32)
    wa1 = W @ a[:F, 0]
    wa2 = W @ a[F:, 0]
    seg2 = np.zeros(SEGL, np.float32); seg2[:F] = wa2; seg2[F] = 1.0
    seg1 = np.zeros(SEGL, np.float32); seg1[:F] = wa1; seg1[F] = 1.0
    row = np.concatenate([np.tile(seg2, BPT), seg1])
    warep = np.broadcast_to(row, (128, XCOLS))
    ident = np.eye(128, dtype=np.float32)
    segbig = (p[:, None] // 16 == (p[None, :] % 8)).astype(np.float32)
    segbig2 = np.concatenate([segbig, segbig], axis=1)
    return np.ascontiguousarray(np.concatenate(
        [warep, W, ident, segbig2], axis=1).astype(bf16))  # [128, XCOLS+512]


def _build_nc(npair=NPAIR, finalize=True):
    import concourse.mybir as mybir
    import concourse.tile as tile
    from concourse import bacc

    fp = mybir.dt.float32
    bf = mybir.dt.bfloat16
    AF = mybir.ActivationFunctionType
    OP = mybir.AluOpType

    nc = bacc.Bacc("TRN2")
    xd = nc.dram_tensor("xd", [2 * npair, 128, XCOLS], bf, kind="ExternalInput")
    cstf = nc.dram_tensor("cstf", [128, 296], fp, kind="ExternalInput")
    cstb = nc.dram_tensor("cstb", [128, XCOLS + 512], bf, kind="ExternalInput")
    yd = nc.dram_tensor("yd", [npair, 128, 256], bf, kind="ExternalOutput")

    with tile.TileContext(nc) as tc:
        with (
            tc.tile_pool(name="const", bufs=1) as constp,
            tc.tile_pool(name="xin", bufs=8) as xin,
            tc.tile_pool(name="small", bufs=4) as small,
            tc.tile_pool(name="big", bufs=4) as big,
            tc.tile_pool(name="scrp", bufs=2) as scrp,
            tc.tile_pool(name="fold", bufs=2) as foldp,
            tc.tile_pool(name="yout", bufs=3) as yout,
            tc.tile_pool(name="ps", bufs=1, space="PSUM") as ps,
        ):
            # ---------------- setup: two DMAs, no device compute ----------
            constsf = constp.tile([128, 296], fp)
            nc.scalar.dma_start(out=constsf, in_=cstf[:, :])
            Cm = constsf[:, 0:128]
            SEG = constsf[:, 128:136]
            E8 = constsf[:, 136:264]
            SEG8_2 = constsf[:, 264:296]

            constsb = constp.tile([128, XCOLS + 512], bf, tag="cb")
            nc.scalar.dma_start(out=constsb, in_=cstb[:, :])
            warep1 = constsb[:, 0:XCOLS]
            W_bf = constsb[:, XCOLS:XCOLS + 128]
            IDENT_bf = constsb[:, XCOLS + 128:XCOLS + 256]
            SEGBIG2_bf = constsb[:, XCOLS + 256:XCOLS + 512]

            # ---------------- software-pipelined pair loop ----------------
            st = {}

            def phase_load(j):
                xall2 = xin.tile([128, XC2], bf, tag="x")
                nc.sync.dma_start(out=xall2[:, 0:XCOLS], in_=xd[2 * j])
                nc.sync.dma_start(out=xall2[:, XCOLS:XC2], in_=xd[2 * j + 1])
                st[j] = {"x": xall2}

            def phase_score(j):
                d = st[j]
                xall2 = d["x"]
                scr = scrp.tile([128, XC2], bf, tag="scr")
                warep_bc = warep1.rearrange(
                    "p (o c) -> p o c", o=1).to_broadcast([128, 2, XCOLS])
                nc.vector.tensor_mul(
                    out=scr.rearrange("p (t c) -> p t c", c=XCOLS),
                    in0=xall2.rearrange("p (t c) -> p t c", c=XCOLS),
                    in1=warep_bc)
                HL = SEGL // 2
                T1 = foldp.tile([128, 34 * HL], bf, tag="t1")
                sv = scr.rearrange("p (s c) -> p s c", c=SEGL)
                nc.vector.tensor_add(
                    out=T1.rearrange("p (s c) -> p s c", c=HL),
                    in0=sv[:, :, 0:HL], in1=sv[:, :, HL:SEGL])
                QL = HL // 2
                T2 = foldp.tile([128, 34 * QL], bf, tag="t2")
                tv = T1.rearrange("p (s c) -> p s c", c=HL)
                nc.vector.tensor_add(
                    out=T2.rearrange("p (s c) -> p s c", c=QL),
                    in0=tv[:, :, 0:QL], in1=tv[:, :, QL:HL])
                s34 = small.tile([128, 34], fp, tag="s34")
                nc.vector.tensor_reduce(
                    out=s34, in_=T2.rearrange("p (s c) -> p s c", c=QL),
                    axis=mybir.AxisListType.X, op=OP.add,
                )
                s34v = s34.rearrange("p (t c) -> p t c", c=BPT + 1)
                Dt2 = small.tile([128, 32], fp, tag="D")
                for t in (0, 1):
                    si_bc = s34[:, t * 17 + 16:t * 17 + 17].rearrange(
                        "p (b o) -> p b o", o=1).to_broadcast([128, K, 1])
                    nc.gpsimd.tensor_mul(
                        out=Dt2[:, 16 * t:16 * t + 16].rearrange(
                            "p (b o) -> p b o", o=1),
                        in0=SEG8_2[:, 16 * t:16 * t + 16].rearrange(
                            "p (b o) -> p b o", o=1),
                        in1=si_bc)
                si_ps = ps.tile([128, 32], fp, tag="si", bufs=1)
                nc.tensor.matmul(si_ps[:, 0:16], lhsT=Cm, rhs=Dt2[:, 0:16],
                                 start=True, stop=True)
                nc.tensor.matmul(si_ps[:, 16:32], lhsT=Cm, rhs=Dt2[:, 16:32],
                                 start=True, stop=True)
                si_s = small.tile([128, 32], fp, tag="si_s")
                nc.scalar.activation(out=si_s, in_=si_ps, func=AF.Copy)
                s2 = small.tile([128, 32], fp, tag="s2")
                nc.gpsimd.tensor_add(
                    out=s2.rearrange("p (t b) -> p t b", b=BPT),
                    in0=s34v[:, :, 0:BPT],
                    in1=si_s.rearrange("p (t b) -> p t b", b=BPT))
                d["s2"] = s2

            def phase_mask(j):
                d = st[j]
                ls = small.tile([128, 32], fp, tag="ls")
                nc.scalar.activation(out=ls, in_=d["s2"], func=AF.Prelu,
                                     alpha=ALPHA)
                exp_s = small.tile([128, 32], fp, tag="exp_s")
                nc.scalar.activation(out=exp_s, in_=ls, func=AF.Exp)
                Z_ps = ps.tile([8, 32], fp, tag="Z", bufs=2)
                nc.tensor.matmul(Z_ps[:, 0:16], lhsT=SEG, rhs=exp_s[:, 0:16],
                                 start=True, stop=True)
                nc.tensor.matmul(Z_ps[:, 16:32], lhsT=SEG, rhs=exp_s[:, 16:32],
                                 start=True, stop=True)
                tz = small.tile([8, 32], fp, tag="tz")
                nc.scalar.activation(out=tz, in_=Z_ps, func=AF.Copy, bias=16.0 * EPS)
                d["p_s"] = exp_s
                d["tz"] = tz

            def phase_recip(j):
                d = st[j]
                RZ = small.tile([8, 32], fp, tag="RZ")
                nc.vector.reciprocal_approx_fast(RZ, d["tz"])
                RZrep_ps = ps.tile([128, 32], fp, tag="RZrep", bufs=2)
                nc.tensor.matmul(RZrep_ps[:, 0:16], lhsT=E8[0:8, :],
                                 rhs=RZ[:, 0:16], start=True, stop=True)
                nc.tensor.matmul(RZrep_ps[:, 16:32], lhsT=E8[0:8, :],
                                 rhs=RZ[:, 16:32], start=True, stop=True)
                RZrep_sb = small.tile([128, 32], fp, tag="RZs")
                nc.scalar.activation(out=RZrep_sb, in_=RZrep_ps, func=AF.Copy)
                d["RZrep"] = RZrep_sb

            def phase_xbar(j):
                d = st[j]
                xall2 = d["x"]
                att = small.tile([128, 32], bf, tag="att")
                nc.gpsimd.tensor_mul(out=att, in0=d["p_s"], in1=d["RZrep"])
                attseg = big.tile([128, 256], bf, tag="attseg")
                att_bc = att.rearrange("p (s o) -> p s o", o=1).to_broadcast(
                    [128, 32, 8])
                nc.vector.tensor_mul(
                    out=attseg.rearrange("p (s j) -> p s j", j=8),
                    in0=SEGBIG2_bf.rearrange("p (s j) -> p s j", j=8),
                    in1=att_bc,
                )
                xbarT_ps = ps.tile([128, 256], fp, tag="mm", bufs=3)
                for t in (0, 1):
                    base = t * XCOLS
                    co = t * 128
                    nc.tensor.matmul(
                        xbarT_ps[:, co:co + 128],
                        lhsT=xall2[:, base + BPT * SEGL:base + BPT * SEGL + F],
                        rhs=IDENT_bf, start=True, stop=False,
                        skip_group_check=True)
                    for b in range(BPT):
                        nc.tensor.matmul(
                            xbarT_ps[:, co + 8 * b:co + 8 * b + 8],
                            lhsT=xall2[:, base + b * SEGL:base + b * SEGL + F],
                            rhs=attseg[:, co + 8 * b:co + 8 * b + 8],
                            start=False, stop=(b == BPT - 1),
                            skip_group_check=True,
                        )
                d["xbarT"] = xbarT_ps

            def phase_out(j):
                d = st[j]
                ST_sb = big.tile([128, 256], bf, tag="ST")
                nc.scalar.activation(out=ST_sb, in_=d["xbarT"], func=AF.Copy)
                zT_ps = ps.tile([128, 256], fp, tag="mm", bufs=3)
                nc.tensor.matmul(zT_ps, lhsT=W_bf, rhs=ST_sb, start=True, stop=True)
                r_sb = big.tile([128, 256], bf, tag="r")
                nc.scalar.activation(out=r_sb, in_=zT_ps, func=AF.Relu)
                e_sb = big.tile([128, 256], bf, tag="e")
                nc.scalar.activation(out=e_sb, in_=zT_ps, func=AF.Exp)
                d["r"] = r_sb
                d["e"] = e_sb

            def phase_store(j):
                d = st[j]
                y_sb = yout.tile([128, 256], bf, tag="y")
                nc.vector.scalar_tensor_tensor(
                    out=y_sb, in0=d["e"], scalar=-1.0, in1=d["r"],
                    op0=OP.add, op1=OP.min,
                )
                nc.sync.dma_start(out=yd[j], in_=y_sb)
                del st[j]

            for r in range(npair + 7):
                if r < npair:
                    phase_load(r)
                if 0 <= r - 7 < npair:
                    phase_store(r - 7)
                if 0 <= r - 6 < npair:
                    phase_out(r - 6)
                if 0 <= r - 5 < npair:
                    phase_xbar(r - 5)
                if 0 <= r - 4 < npair:
                    phase_recip(r - 4)
                if 0 <= r - 3 < npair:
                    phase_mask(r - 3)
                if 0 <= r - 2 < npair:
                    phase_score(r - 2)

    if finalize:
        nc.finalize()
    return nc


def _get_nc():
    if NPAIR not in _NC_CACHE:
        _NC_CACHE[NPAIR] = _build_nc(NPAIR)
    return _NC_CACHE[NPAIR]


def _shard_inputs(orignal_x, x, adj, W, a, ncores=NCORES):
    import ml_dtypes
    bf16 = ml_dtypes.bfloat16
    f32 = np.float32
    rpc = RPC
    x = np.asarray(x, f32)
    x0 = np.asarray(orignal_x, f32)
    adj = np.asarray(adj, np.int32)
    cf = _consts_f32_np()
    cb = _consts_bf16_np(W, a)
    n = x.shape[0]

    in_maps = []
    for c in range(ncores):
        lo = c * rpc
        hi = min((c + 1) * rpc, n)
        rows = max(hi - lo, 0)
        xc = x[lo:hi]
        x0c = x0[lo:hi]
        adjc = adj[lo:hi]
        if rows < rpc:
            pad = rpc - rows
            xc = np.concatenate([xc, np.zeros((pad, K, F), f32)])
            x0c = np.concatenate([x0c, np.zeros((pad, F), f32)])
            adjc = np.concatenate([adjc, np.zeros((pad, K), np.int32)])
        xdev = np.zeros((NTILES, 128, XCOLS), bf16)
        xs = xdev.reshape(NTILES, 128, BPT + 1, SEGL)
        xs[:, :, :BPT, :F] = xc.reshape(NTILES, BPT, 128, F).transpose(
            0, 2, 1, 3).astype(bf16)
        mb = (MASKB * (1 - adjc)).astype(f32).reshape(
            NTILES, BPT, 128).transpose(0, 2, 1)
        xs[:, :, :BPT, F] = mb.astype(bf16)
        xs[:, :, BPT, :F] = x0c.reshape(NTILES, 128, F).astype(bf16)
        in_maps.append({
            "xd": xdev,
            "cstf": cf,
            "cstb": cb,
        })
    return in_maps


_LAST_RESULTS = None


def kernel(orignal_x, x, adj, W, a):
    import os
    import sys
    os.environ.setdefault("JAX_PLATFORMS", "")
    try:
        from concourse.bass_utils import run_bass_kernel_spmd
    except ImportError:
        sys.path.insert(0, "/opt/trn_rl_repo")
        from concourse.bass_utils import run_bass_kernel_spmd

    global _LAST_RESULTS
    nc = _get_nc()
    in_maps = _shard_inputs(orignal_x, x, adj, W, a)
    res = run_bass_kernel_spmd(nc, in_maps, list(range(NCORES)))
    _LAST_RESULTS = res
    # yd [NPAIR, 128 f_out, 2*128] in transposed (zT) layout
    parts = []
    for r in res.results:
        yT = np.asarray(r["yd"], np.float32)           # [25, 128, 256]
        yc = yT.reshape(NPAIR, 128, 2, 128).transpose(0, 2, 3, 1)
        parts.append(yc.reshape(RPC, F))
    y = np.concatenate(parts, axis=0)
    return np.ascontiguousarray(y[:N])


# revision 31
# speedup vs baseline: 1.0025x; 1.0025x over previous
"""GAT attention kernel for 8 trn2 NeuronCores (Bass/Tile), bf16 v7.

Math (restructured from the reference to avoid materializing h_j):
    wa1 = W @ a1, wa2 = W @ a2                      (host, once, O(F^2))
    s[n,k]  = x[n,k]·wa2 - 500*(1-adj[n,k])         (mask folded into score)
    s2      = s + si,  si = x0[n]·wa1
    e       = exp(leaky_relu(s2, 0.2))              (masked: exp(~-100) ~ 0,
                                                     same as reference -9e15 mask)
    att     = e / (sum_k e + 16*EPS)
    xbar[n] = sum_k att[n,k] * x[n,k,:]
    out     = elu((xbar + x0) @ W)                  (h_prime + h = (xbar + x0)@W)
    elu(z)  = min(exp(z) - 1, relu(z))

Layout: node dim N padded 50000 -> 51200 = 8 cores * 25 pairs * 256 rows.
TWO 128-row tiles are processed per pipeline step ("pair") to amortize the
~151-cycle DVE and ~222-cycle ACT fixed costs.  Per tile, 17 segments of 132
bf16 elems: 16x [x_block(128) | -500(1-adj) | 0 0 0] (s-layout nk blocks) +
[x0 | 0...].  Scores: one 2x TT product against a precomposed weights tile,
two 2x bf16 fold-adds (132->66->33), then the 1x segmented reduce on a third
of the data -- ~25% less DVE time than reducing directly.
Final matmul is transposed (lhsT = W, rhs = xbarT(+x0T) pair-wide): one
256-col matmul per pair; output lands [f_out, n] and the host transposes.

Per pair:
  DVE : prod TT, fold1, fold2, segmented reduce, attseg = SEGBIG*att,
        recip_fast, y = min(E-1, R)
  PE  : 2x si scatter (Cm), 2x Z (SEG), 2x RZrep (E8), 2x x0^T ident +
        32 xbar matmuls (PSUM), final W^T-form matmul (256 cols)
  ACT : si_s, Prelu, Exp, tz (+16eps), RZrep copy, ST copy, R, E
  GPS : 2x Dt (si bcast mask), s2 = s+si_s, att = e*RZ
"""

import numpy as np

N, K, F = 50000, 16, 128
ALPHA = 0.2
NCORES = 8
TILE = 128
NTILES = 50                  # padded to even
NPAIR = NTILES // 2
RPC = TILE * NTILES          # rows per core = 6400
BPT = K                      # nk-blocks per tile = 16
SEGL = F + 4                 # 128 features + mask bias + 3 pad (fold-friendly)
XCOLS = (BPT + 1) * SEGL     # 17 segments of 132 = 2244
XC2 = 2 * XCOLS              # pair row = 4488
EPS = 1e-12
MASKB = -500.0

_NC_CACHE = {}


def _consts_f32_np():
    p = np.arange(128)
    j8 = np.arange(8)
    b16 = np.arange(16)
    # C[n, q] = 1 iff n%8 == q//16   (si scatter: out[q,b] = si[8b + q//16])
    Cm = (p[:, None] % 8 == p[None, :] // 16).astype(np.float32)
    # SEG[q, j] = 1 iff q//16 == j   [128, 8]
    seg = (p[:, None] // 16 == j8[None, :]).astype(np.float32)
    # E8 rows 0..8: E8[j, q] = 1 iff q//16 == j (used as lhsT [8,128])
    e8 = ((p[:, None] < 8) & (p[None, :] // 16 == p[:, None])).astype(np.float32)
    # SEG8[n, b] = 1 iff n//8 == b   [128, 16], tiled x2 for the pair
    seg8 = (p[:, None] // 8 == b16[None, :]).astype(np.float32)
    seg8_2 = np.concatenate([seg8, seg8], axis=1)
    return np.ascontiguousarray(
        np.concatenate([Cm, seg, e8, seg8_2], axis=1))  # [128, 296]


def _consts_bf16_np(W, a):
    import ml_dtypes
    bf16 = ml_dtypes.bfloat16
    p = np.arange(128)
    W = np.asarray(W, np.float32)
    a = np.asarray(a, np.float32)
    wa1 = W @ a[:F, 0]
    wa2 = W @ a[F:, 0]
    seg2 = np.zeros(SEGL, np.float32); seg2[:F] = wa2; seg2[F] = 1.0
    seg1 = np.zeros(SEGL, np.float32); seg1[:F] = wa1; seg1[F] = 1.0
    row = np.concatenate([np.tile(seg2, BPT), seg1])
    warep = np.broadcast_to(row, (128, XCOLS))
    ident = np.eye(128, dtype=np.float32)
    segbig = (p[:, None] // 16 == (p[None, :] % 8)).astype(np.float32)
    segbig2 = np.concatenate([segbig, segbig], axis=1)
    return np.ascontiguousarray(np.concatenate(
        [warep, W, ident, segbig2], axis=1).astype(bf16))  # [128, XCOLS+512]


def _build_nc(npair=NPAIR, finalize=True):
    import concourse.mybir as mybir
    import concourse.tile as tile
    from concourse import bacc

    fp = mybir.dt.float32
    bf = mybir.dt.bfloat16
    AF = mybir.ActivationFunctionType
    OP = mybir.AluOpType

    nc = bacc.Bacc("TRN2")
    xd = nc.dram_tensor("xd", [2 * npair, 128, XCOLS], bf, kind="ExternalInput")
    cstf = nc.dram_tensor("cstf", [128, 296], fp, kind="ExternalInput")
    cstb = nc.dram_tensor("cstb", [128, XCOLS + 512], bf, kind="ExternalInput")
    yd = nc.dram_tensor("yd", [npair, 128, 256], bf, kind="ExternalOutput")

    with tile.TileContext(nc) as tc:
        with (
            tc.tile_pool(name="const", bufs=1) as constp,
            tc.tile_pool(name="xin", bufs=8) as xin,
            tc.tile_pool(name="small", bufs=4) as small,
            tc.tile_pool(name="big", bufs=4) as big,
            tc.tile_pool(name="scrp", bufs=2) as scrp,
            tc.tile_pool(name="fold", bufs=2) as foldp,
            tc.tile_pool(name="yout", bufs=3) as yout,
            tc.tile_pool(name="ps", bufs=1, space="PSUM") as ps,
        ):
            # ---------------- setup: two DMAs, no device compute ----------
            constsf = constp.tile([128, 296], fp)
            nc.scalar.dma_start(out=constsf, in_=cstf[:, :])
            Cm = constsf[:, 0:128]
            SEG = constsf[:, 128:136]
            E8 = constsf[:, 136:264]
            SEG8_2 = constsf[:, 264:296]

            constsb = constp.tile([128, XCOLS + 512], bf, tag="cb")
            nc.scalar.dma_start(out=constsb, in_=cstb[:, :])
            warep1 = constsb[:, 0:XCOLS]
            W_bf = constsb[:, XCOLS:XCOLS + 128]
            IDENT_bf = constsb[:, XCOLS + 128:XCOLS + 256]
            SEGBIG2_bf = constsb[:, XCOLS + 256:XCOLS + 512]

            # ---------------- software-pipelined pair loop ----------------
            st = {}

            def phase_load(j):
                xall2 = xin.tile([128, XC2], bf, tag="x")
                nc.sync.dma_start(out=xall2[:, 0:XCOLS], in_=xd[2 * j])
                nc.sync.dma_start(out=xall2[:, XCOLS:XC2], in_=xd[2 * j + 1])
                st[j] = {"x": xall2}

            def phase_score(j):
                d = st[j]
                xall2 = d["x"]
                scr = scrp.tile([128, XC2], bf, tag="scr")
                warep_bc = warep1.rearrange(
                    "p (o c) -> p o c", o=1).to_broadcast([128, 2, XCOLS])
                nc.vector.tensor_mul(
                    out=scr.rearrange("p (t c) -> p t c", c=XCOLS),
                    in0=xall2.rearrange("p (t c) -> p t c", c=XCOLS),
                    in1=warep_bc)
                HL = SEGL // 2
                T1 = foldp.tile([128, 34 * HL], bf, tag="t1")
                sv = scr.rearrange("p (s c) -> p s c", c=SEGL)
                nc.vector.tensor_add(
                    out=T1.rearrange("p (s c) -> p s c", c=HL),
                    in0=sv[:, :, 0:HL], in1=sv[:, :, HL:SEGL])
                QL = HL // 2
                T2 = foldp.tile([128, 34 * QL], bf, tag="t2")
                tv = T1.rearrange("p (s c) -> p s c", c=HL)
                nc.vector.tensor_add(
                    out=T2.rearrange("p (s c) -> p s c", c=QL),
                    in0=tv[:, :, 0:QL], in1=tv[:, :, QL:HL])
                s34 = small.tile([128, 34], fp, tag="s34")
                nc.vector.tensor_reduce(
                    out=s34, in_=T2.rearrange("p (s c) -> p s c", c=QL),
                    axis=mybir.AxisListType.X, op=OP.add,
                )
                s34v = s34.rearrange("p (t c) -> p t c", c=BPT + 1)
                Dt2 = small.tile([128, 32], fp, tag="D")
                for t in (0, 1):
                    si_bc = s34[:, t * 17 + 16:t * 17 + 17].rearrange(
                        "p (b o) -> p b o", o=1).to_broadcast([128, K, 1])
                    nc.gpsimd.tensor_mul(
                        out=Dt2[:, 16 * t:16 * t + 16].rearrange(
                            "p (b o) -> p b o", o=1),
                        in0=SEG8_2[:, 16 * t:16 * t + 16].rearrange(
                            "p (b o) -> p b o", o=1),
                        in1=si_bc)
                si_ps = ps.tile([128, 32], fp, tag="si", bufs=1)
                nc.tensor.matmul(si_ps[:, 0:16], lhsT=Cm, rhs=Dt2[:, 0:16],
                                 start=True, stop=True)
                nc.tensor.matmul(si_ps[:, 16:32], lhsT=Cm, rhs=Dt2[:, 16:32],
                                 start=True, stop=True)
                si_s = small.tile([128, 32], fp, tag="si_s")
                nc.scalar.activation(out=si_s, in_=si_ps, func=AF.Copy)
                s2 = small.tile([128, 32], fp, tag="s2")
                nc.gpsimd.tensor_add(
                    out=s2.rearrange("p (t b) -> p t b", b=BPT),
                    in0=s34v[:, :, 0:BPT],
                    in1=si_s.rearrange("p (t b) -> p t b", b=BPT))
                d["s2"] = s2

            def phase_mask(j):
                d = st[j]
                ls = small.tile([128, 32], fp, tag="ls")
                nc.scalar.activation(out=ls, in_=d["s2"], func=AF.Prelu,
                                     alpha=ALPHA)
                exp_s = small.tile([128, 32], fp, tag="exp_s")
                nc.scalar.activation(out=exp_s, in_=ls, func=AF.Exp)
                Z_ps = ps.tile([8, 32], fp, tag="Z", bufs=2)
                nc.tensor.matmul(Z_ps[:, 0:16], lhsT=SEG, rhs=exp_s[:, 0:16],
                                 start=True, stop=True)
                nc.tensor.matmul(Z_ps[:, 16:32], lhsT=SEG, rhs=exp_s[:, 16:32],
                                 start=True, stop=True)
                tz = small.tile([8, 32], fp, tag="tz")
                nc.scalar.activation(out=tz, in_=Z_ps, func=AF.Copy, bias=16.0 * EPS)
                d["p_s"] = exp_s
                d["tz"] = tz

            def phase_recip(j):
                d = st[j]
                RZ = small.tile([8, 32], fp, tag="RZ")
                nc.vector.reciprocal_approx_fast(RZ, d["tz"])
                RZrep_ps = ps.tile([128, 32], fp, tag="RZrep", bufs=2)
                nc.tensor.matmul(RZrep_ps[:, 0:16], lhsT=E8[0:8, :],
                                 rhs=RZ[:, 0:16], start=True, stop=True)
                nc.tensor.matmul(RZrep_ps[:, 16:32], lhsT=E8[0:8, :],
                                 rhs=RZ[:, 16:32], start=True, stop=True)
                RZrep_sb = small.tile([128, 32], fp, tag="RZs")
                nc.scalar.activation(out=RZrep_sb, in_=RZrep_ps, func=AF.Copy)
                d["RZrep"] = RZrep_sb

            def phase_xbar(j):
                d = st[j]
                xall2 = d["x"]
                att = small.tile([128, 32], bf, tag="att")
                nc.gpsimd.tensor_mul(out=att, in0=d["p_s"], in1=d["RZrep"])
                attseg = big.tile([128, 256], bf, tag="attseg")
                att_bc = att.rearrange("p (s o) -> p s o", o=1).to_broadcast(
                    [128, 32, 8])
                nc.vector.tensor_mul(
                    out=attseg.rearrange("p (s j) -> p s j", j=8),
                    in0=SEGBIG2_bf.rearrange("p (s j) -> p s j", j=8),
                    in1=att_bc,
                )
                xbarT_ps = ps.tile([128, 256], fp, tag="mm", bufs=3)
                for t in (0, 1):
                    base = t * XCOLS
                    co = t * 128
                    nc.tensor.matmul(
                        xbarT_ps[:, co:co + 128],
                        lhsT=xall2[:, base + BPT * SEGL:base + BPT * SEGL + F],
                        rhs=IDENT_bf, start=True, stop=False,
                        skip_group_check=True)
                    for b in range(BPT):
                        nc.tensor.matmul(
                            xbarT_ps[:, co + 8 * b:co + 8 * b + 8],
                            lhsT=xall2[:, base + b * SEGL:base + b * SEGL + F],
                            rhs=attseg[:, co + 8 * b:co + 8 * b + 8],
                            start=False, stop=(b == BPT - 1),
                            skip_group_check=True,
                        )
                d["xbarT"] = xbarT_ps

            def phase_out(j):
                d = st[j]
                ST_sb = big.tile([128, 256], bf, tag="ST")
                nc.scalar.activation(out=ST_sb, in_=d["xbarT"], func=AF.Copy)
                zT_ps = ps.tile([128, 256], fp, tag="mm", bufs=3)
                nc.tensor.matmul(zT_ps, lhsT=W_bf, rhs=ST_sb, start=True, stop=True)
                r_sb = big.tile([128, 256], bf, tag="r")
                nc.scalar.activation(out=r_sb, in_=zT_ps, func=AF.Relu)
                e_sb = big.tile([128, 256], bf, tag="e")
                nc.scalar.activation(out=e_sb, in_=zT_ps, func=AF.Exp)
                d["r"] = r_sb
                d["e"] = e_sb

            def phase_store(j):
                d = st[j]
                y_sb = yout.tile([128, 256], bf, tag="y")
                nc.vector.scalar_tensor_tensor(
                    out=y_sb, in0=d["e"], scalar=-1.0, in1=d["r"],
                    op0=OP.add, op1=OP.min,
                )
                nc.sync.dma_start(out=yd[j], in_=y_sb)
                del st[j]

            for r in range(npair + 6):
                if r < npair:
                    phase_load(r)
                if 0 <= r - 6 < npair:
                    phase_store(r - 6)
                if 0 <= r - 5 < npair:
                    phase_out(r - 5)
                if 0 <= r - 4 < npair:
                    phase_xbar(r - 4)
                if 0 <= r - 3 < npair:
                    phase_recip(r - 3)
                if 0 <= r - 2 < npair:
                    phase_mask(r - 2)
                if 0 <= r - 1 < npair:
                    phase_score(r - 1)

    if finalize:
        nc.finalize()
    return nc


def _get_nc():
    if NPAIR not in _NC_CACHE:
        _NC_CACHE[NPAIR] = _build_nc(NPAIR)
    return _NC_CACHE[NPAIR]


def _shard_inputs(orignal_x, x, adj, W, a, ncores=NCORES):
    import ml_dtypes
    bf16 = ml_dtypes.bfloat16
    f32 = np.float32
    rpc = RPC
    x = np.asarray(x, f32)
    x0 = np.asarray(orignal_x, f32)
    adj = np.asarray(adj, np.int32)
    cf = _consts_f32_np()
    cb = _consts_bf16_np(W, a)
    n = x.shape[0]

    in_maps = []
    for c in range(ncores):
        lo = c * rpc
        hi = min((c + 1) * rpc, n)
        rows = max(hi - lo, 0)
        xc = x[lo:hi]
        x0c = x0[lo:hi]
        adjc = adj[lo:hi]
        if rows < rpc:
            pad = rpc - rows
            xc = np.concatenate([xc, np.zeros((pad, K, F), f32)])
            x0c = np.concatenate([x0c, np.zeros((pad, F), f32)])
            adjc = np.concatenate([adjc, np.zeros((pad, K), np.int32)])
        xdev = np.zeros((NTILES, 128, XCOLS), bf16)
        xs = xdev.reshape(NTILES, 128, BPT + 1, SEGL)
        xs[:, :, :BPT, :F] = xc.reshape(NTILES, BPT, 128, F).transpose(
            0, 2, 1, 3).astype(bf16)
        mb = (MASKB * (1 - adjc)).astype(f32).reshape(
            NTILES, BPT, 128).transpose(0, 2, 1)
        xs[:, :, :BPT, F] = mb.astype(bf16)
        xs[:, :, BPT, :F] = x0c.reshape(NTILES, 128, F).astype(bf16)
        in_maps.append({
            "xd": xdev,
            "cstf": cf,
            "cstb": cb,
        })
    return in_maps


_LAST_RESULTS = None


def kernel(orignal_x, x, adj, W, a):
    import os
    import sys
    os.environ.setdefault("JAX_PLATFORMS", "")
    try:
        from concourse.bass_utils import run_bass_kernel_spmd
    except ImportError:
        sys.path.insert(0, "/opt/trn_rl_repo")
        from concourse.bass_utils import run_bass_kernel_spmd

    global _LAST_RESULTS
    nc = _get_nc()
    in_maps = _shard_inputs(orignal_x, x, adj, W, a)
    res = run_bass_kernel_spmd(nc, in_maps, list(range(NCORES)))
    _LAST_RESULTS = res
    # yd [NPAIR, 128 f_out, 2*128] in transposed (zT) layout
    parts = []
    for r in res.results:
        yT = np.asarray(r["yd"], np.float32)           # [25, 128, 256]
        yc = yT.reshape(NPAIR, 128, 2, 128).transpose(0, 2, 3, 1)
        parts.append(yc.reshape(RPC, F))
    y = np.concatenate(parts, axis=0)
    return np.ascontiguousarray(y[:N])
